# revision 1
# baseline (speedup 1.0000x reference)
"""DeformConv3D on 8 TRN2 cores: H-sharded, dense 5-tap tent-weight gather.

Per core (h-band of 12 output rows + halos):
  P1: offset conv (27 taps, K=64 matmuls accumulated in PSUM) -> off scratch DRAM
  P2: trilinear gather as separable 5-tap tent-weighted sums on DVE
      (one (b,c) plane per partition; all shifts are AP offsets into a
       padded per-plane window; tent weights vanish outside the clamp range
       so padded reads are weight-zero)
  P3: main conv + bias -> output h-band
"""
import sys, os
import numpy as np
from contextlib import ExitStack

sys.path.insert(0, "/opt/trn_rl_repo")
from concourse import bass, bacc, tile, mybir
from concourse.bass_utils import run_bass_kernel_spmd

F32 = mybir.dt.float32
BF16 = mybir.dt.bfloat16
ALU = mybir.AluOpType
AF = mybir.ActivationFunctionType

B, C, L, H, W = 2, 64, 16, 96, 96
CO1, CO2 = 192, 64
NCORES = 8
HB = H // NCORES       # 12 output rows per core
HW_ROWS = 20           # x window rows per core: [12k-4, 12k+16)
HG = 14                # gather rows per core: [12k-1, 12k+13)
NPP = HG * W           # 1344 gather outputs per (plane, l)
ZPAD, XPAD = 20, 100   # gather window padded dims (taps +-2)
WIN = HW_ROWS * ZPAD * XPAD
CZP, CXP = 18, 98      # conv window padded dims (taps +-1)
TAPS = (-2, -1, 0, 1, 2)

_nc1_cache = None
_nc2_cache = None


def build_program1():
    nc = bacc.Bacc("TRN2", target_bir_lowering=False, debug=False, num_devices=NCORES)
    xwin = nc.dram_tensor("xwin", [B, C, L, HW_ROWS, W], F32, kind="ExternalInput").ap()
    w_off = nc.dram_tensor("w_off", [27, C, CO1], F32, kind="ExternalInput").ap()
    off_scr = nc.dram_tensor("off_band", [B, CO1, L, HG, W], F32, kind="ExternalOutput").ap()
    ctx = ExitStack()
    with tile.TileContext(nc) as tc:
        # ---------------- Phase 1: offset conv ----------------
        with tc.tile_pool(name="p1", bufs=1) as p1, \
             tc.tile_pool(name="p1ps", bufs=2, space="PSUM") as p1ps, \
             tc.tile_pool(name="p1o", bufs=3) as p1o:
            wofft = p1.tile([C, 27, CO1], F32)
            nc.sync.dma_start(wofft[:], w_off.rearrange("t c m -> c t m"))
            for b in range(B):
                xc = p1.tile([C, CZP, HW_ROWS, CXP], F32, tag="xc")
                nc.vector.memset(xc[:].rearrange("c z y x -> c (z y x)"), 0.0)
                for z in range(L):
                    nc.sync.dma_start(xc[:, 1 + z, :, 1:W + 1], xwin[b, :, z])
                for l in range(L):
                    for hc0, hcn in ((0, 5), (5, 5), (10, 4)):
                        nmm = hcn * W
                        for m0, mw in ((0, 128), (128, 64)):
                            ps = p1ps.tile([128, 480], F32, tag="ps1")
                            for t in range(27):
                                dz, rem = divmod(t, 9)
                                dy, dx = divmod(rem, 3)
                                rhs = xc[:, l + dz,
                                         2 + hc0 + dy:2 + hc0 + dy + hcn,
                                         dx:dx + W]
                                nc.tensor.matmul(
                                    ps[:mw, :nmm], wofft[:, t, m0:m0 + mw],
                                    rhs, start=(t == 0), stop=(t == 26))
                            ob = p1o.tile([128, 480], F32, tag="ob1")
                            nc.vector.tensor_copy(ob[:mw, :nmm], ps[:mw, :nmm])
                            nc.sync.dma_start(
                                off_scr[b, m0:m0 + mw, l, hc0:hc0 + hcn, :]
                                .rearrange("m h x -> m (h x)"),
                                ob[:mw, :nmm])

    nc.finalize()
    return nc


def build_program2():
    nc = bacc.Bacc("TRN2", target_bir_lowering=False, debug=False, num_devices=NCORES)
    xwin = nc.dram_tensor("xwin", [B, C, L, HW_ROWS, W], BF16, kind="ExternalInput").ap()
    w_conv = nc.dram_tensor("w_conv", [27, C, CO2], F32, kind="ExternalInput").ap()
    b_conv = nc.dram_tensor("b_conv", [CO2, 1], F32, kind="ExternalInput").ap()
    offs = nc.dram_tensor("offs", [128, 3, L, NPP], F32, kind="ExternalInput").ap()
    grids = nc.dram_tensor("grids", [128, 1, NPP], F32, kind="ExternalInput").ap()
    out_ext = nc.dram_tensor("out", [B, CO2, L, HB, W], F32, kind="ExternalOutput").ap()
    def_scr = nc.dram_tensor("def_scr", [B, C, L, HG, W], F32).ap()
    ctx = ExitStack()
    with tile.TileContext(nc) as tc:
        # ---------------- Phase 2: tent gather ----------------
        with tc.tile_pool(name="p2w", bufs=1) as p2w, \
             tc.tile_pool(name="p2", bufs=1) as p2:
            win = p2w.tile([128, HW_ROWS, ZPAD, XPAD], BF16)
            nc.vector.memset(win[:].rearrange("p y z x -> p (y z x)"), 0.0)
            for b in range(B):
                for z in range(L):
                    nc.sync.dma_start(
                        win[64 * b:64 * b + 64, :, 2 + z, 2:W + 2],
                        xwin[b, :, z])
            gr = p2w.tile([128, 1, NPP], F32)
            nc.sync.dma_start(gr[:], grids)
            zbias = p2w.tile([128, 1], F32)
            nc.vector.memset(zbias[:], 0.0)

            for l in range(L):
                offc = p2.tile([128, 3, NPP], F32, tag="off")
                nc.sync.dma_start(offc[:], offs[:, :, l, :])
                az = offc[:, 0]
                ay = offc[:, 1]
                ax = offc[:, 2]

                # tent weights lam[dim][tap] = relu(1 - |a - t|)  (bf16)
                tneg = p2.tile([128, NPP], F32, tag="tneg")
                tpos = p2.tile([128, NPP], F32, tag="tpos")

                def tents(a, dst_tag, taps):
                    row = []
                    for t in taps:
                        nc.vector.tensor_scalar(tpos[:], a, 1.0 - float(t), None, ALU.add)
                        nc.vector.tensor_scalar(tneg[:], a, -1.0, 1.0 + float(t), ALU.mult, ALU.add)
                        nc.vector.tensor_tensor(tpos[:], tpos[:], tneg[:], ALU.min)
                        lt = p2.tile([128, NPP], BF16, tag=f"{dst_tag}_{t}")
                        nc.scalar.activation(lt[:], tpos[:], AF.Relu, bias=zbias[:])
                        row.append(lt)
                    return row

                lamx = tents(ax, "lamx", TAPS)
                lamy = tents(ay, "lamy", TAPS)

                acc = p2.tile([128, NPP], F32, tag="acc")
                tmpi = p2.tile([128, NPP], F32, tag="tmpi")
                tmpb = p2.tile([128, NPP], F32, tag="tmpb")
                prod = p2.tile([128, NPP], BF16, tag="prod")
                lam = [None, lamy, lamx]
                for iz, sz in enumerate(TAPS):
                    lamz = tents(az, "lamz", (sz,))[0]
                    for iy, sy in enumerate(TAPS):
                        for ix, sx in enumerate(TAPS):
                            v = win[:, 3 + sy:3 + sy + HG,
                                    l + 2 + sz,
                                    2 + sx:2 + sx + W]
                            if ix == 0:
                                nc.vector.tensor_tensor(tmpi[:], lam[2][0][:], v, ALU.mult)
                            else:
                                nc.vector.tensor_tensor(prod[:], lam[2][ix][:], v, ALU.mult)
                                nc.vector.tensor_tensor(tmpi[:], tmpi[:], prod[:], ALU.add)
                        if iy == 0:
                            nc.vector.tensor_tensor(tmpb[:], lam[1][0][:], tmpi[:], ALU.mult)
                        else:
                            nc.vector.tensor_tensor(tmpi[:], lam[1][iy][:], tmpi[:], ALU.mult)
                            nc.vector.tensor_tensor(tmpb[:], tmpb[:], tmpi[:], ALU.add)
                    if iz == 0:
                        nc.vector.tensor_tensor(acc[:], lamz[:], tmpb[:], ALU.mult)
                    else:
                        nc.vector.tensor_tensor(tmpb[:], lamz[:], tmpb[:], ALU.mult)
                        nc.vector.tensor_tensor(acc[:], acc[:], tmpb[:], ALU.add)
                # zero rows whose global h is outside [0, 96)
                nc.vector.tensor_tensor(acc[:], acc[:], gr[:, 0], ALU.mult)
                for b in range(B):
                    nc.sync.dma_start(
                        def_scr[b, :, l].rearrange("c h x -> c (h x)"),
                        acc[64 * b:64 * b + 64, :])

        # ---------------- Phase 3: main conv ----------------
        with tc.tile_pool(name="p3", bufs=1) as p3, \
             tc.tile_pool(name="p3ps", bufs=2, space="PSUM") as p3ps, \
             tc.tile_pool(name="p3o", bufs=3) as p3o:
            wct = p3.tile([C, 27, CO2], F32)
            nc.sync.dma_start(wct[:], w_conv.rearrange("t c m -> c t m"))
            bct = p3.tile([CO2, 1], F32)
            nc.sync.dma_start(bct[:], b_conv)
            for b in range(B):
                dc = p3.tile([C, CZP, HG + 2, CXP], F32, tag="dc")
                nc.vector.memset(dc[:].rearrange("c z y x -> c (z y x)"), 0.0)
                for z in range(L):
                    nc.sync.dma_start(dc[:, 1 + z, 1:HG + 1, 1:W + 1], def_scr[b, :, z])
                for l in range(L):
                    for hc0, hcn in ((0, 5), (5, 5), (10, 2)):
                        nmm = hcn * W
                        ps = p3ps.tile([CO2, 480], F32, tag="ps3")
                        for t in range(27):
                            dz, rem = divmod(t, 9)
                            dy, dx = divmod(rem, 3)
                            # out row r=4+hc0+j -> dc y index r+dy-3
                            rhs = dc[:, l + dz,
                                     1 + hc0 + dy:1 + hc0 + dy + hcn,
                                     dx:dx + W]
                            nc.tensor.matmul(
                                ps[:, :nmm], wct[:, t, :],
                                rhs, start=(t == 0), stop=(t == 26))
                        ob = p3o.tile([CO2, 480], F32, tag="ob3")
                        nc.vector.tensor_scalar(ob[:, :nmm], ps[:, :nmm], bct[:], None, ALU.add)
                        nc.sync.dma_start(
                            out_ext[b, :, l, hc0:hc0 + hcn, :]
                            .rearrange("m h x -> m (h x)"),
                            ob[:, :nmm])
    nc.finalize()
    return nc


def kernel(x, w_off, w_conv, b_conv):
    global _nc1_cache, _nc2_cache
    x = np.asarray(x, dtype=np.float32)
    w_off = np.asarray(w_off, dtype=np.float32)
    w_conv = np.asarray(w_conv, dtype=np.float32)
    b_conv = np.asarray(b_conv, dtype=np.float32)

    if _nc1_cache is None:
        _nc1_cache = build_program1()
        _nc2_cache = build_program2()

    xp = np.zeros((B, C, L, H + 8, W), np.float32)
    xp[:, :, :, 4:4 + H, :] = x
    wofft = np.ascontiguousarray(
        w_off.reshape(CO1, C, 27).transpose(2, 1, 0))        # [27, C, CO1]
    wct = np.ascontiguousarray(
        w_conv.reshape(CO2, C, 27).transpose(2, 1, 0))       # [27, C, CO2]
    bc = np.ascontiguousarray(b_conv.reshape(CO2, 1))

    xwins = [np.ascontiguousarray(xp[:, :, :, 12 * k:12 * k + HW_ROWS, :])
             for k in range(NCORES)]
    import ml_dtypes
    xwins_bf = [w.astype(ml_dtypes.bfloat16) for w in xwins]
    in1 = [{"xwin": xwins[k], "w_off": wofft} for k in range(NCORES)]
    res1 = run_bass_kernel_spmd(_nc1_cache, in1, list(range(NCORES)))

    # reassemble full off field from per-core bands (band rows = 12k-1..12k+13)
    off_full = np.empty((B, CO1, L, H, W), np.float32)
    for k in range(NCORES):
        band = res1.results[k]["off_band"]
        off_full[:, :, :, 12 * k:12 * k + HB, :] = band[:, :, :, 1:1 + HB, :]
    # contiguous-view scramble: plane (b,c) offsets at spatial p, comp k =
    # flat element 3p+k of its 3-channel block. Per (l, h) row that is a
    # contiguous 288-float run, so a padded reshape + slice does it all.
    tri = off_full.reshape(B * C, L, 3 * H * W)
    trip = np.zeros((B * C, L, 3 * (H + 2) * W), np.float32)
    trip[:, :, 3 * W:3 * (H + 1) * W] = tri            # one pad row each side
    trip = trip.reshape(B * C, L, H + 2, W * 3)
    in2 = []
    gy = np.repeat(np.arange(HG, dtype=np.float32) + 3.0, W)
    gx = np.tile(np.arange(W, dtype=np.float32), HG)
    lgrid = np.arange(L, dtype=np.float32)[None, None, :, None]
    for k in range(NCORES):
        seg = trip[:, :, 12 * k:12 * k + HG, :]        # rows 12k-1..12k+13
        offs = np.ascontiguousarray(
            seg.reshape(128, L, HG * W, 3).transpose(0, 3, 1, 2))
        # displacements a = clamp(off + grid) - grid, computed on host
        hglobf = np.repeat(np.arange(HG, dtype=np.float32) + (12 * k - 1), W)
        offs[:, 0] = np.clip(offs[:, 0] + lgrid[0], 0.0, 15.0) - lgrid[0]
        offs[:, 1] = (np.clip(offs[:, 1] + hglobf[None, None, :], 0.0, 95.0)
                      - hglobf[None, None, :])
        offs[:, 2] = (np.clip(offs[:, 2] + gx[None, None, :], 0.0, 95.0)
                      - gx[None, None, :])
        hglob = np.repeat(np.arange(HG) + (12 * k - 1), W)
        ymask = ((hglob >= 0) & (hglob < H)).astype(np.float32)
        grids = np.broadcast_to(ymask[None, None], (128, 1, NPP)).copy()
        in2.append({
            "xwin": xwins_bf[k], "w_conv": wct, "b_conv": bc,
            "offs": offs,
            "grids": grids,
        })
    res2 = run_bass_kernel_spmd(_nc2_cache, in2, list(range(NCORES)))
    out = np.empty((B, CO2, L, H, W), np.float32)
    for k in range(NCORES):
        out[:, :, :, 12 * k:12 * k + HB, :] = res2.results[k]["out"]
    return out



# revision 11
# speedup vs baseline: 2.1120x; 2.1120x over previous
"""DeformConv3D on 8 TRN2 cores — fused single-launch kernel.

H-sharded (12 output rows/core). The torch contiguous-view offset scramble
means core k's gather offsets come from scattered 3-row runs of the offset
conv's output volume; each core computes its offset conv DIRECTLY at those
scrambled rows using host-assembled per-triplet input slabs (per-core
variation lives in input VALUES only, so the SPMD program is uniform):

  P1: offset conv at scrambled triplet rows (bf16 matmuls, dy-tap pairs
      packed into k=128) -> offc SBUF
  P2: trilinear gather as dense 5-tap tent-weight sums on DVE
      (fused broadcast/reduce ops, bf16)
  P3: main conv + bias (bf16 matmuls, dy-pairs k=128) from def_scr DRAM
"""
import sys
import numpy as np

sys.path.insert(0, "/opt/trn_rl_repo")
from concourse import bass, bacc, tile, mybir
from concourse.ap import AP
from concourse.bass import broadcast_tensor_aps
from concourse.bass_utils import run_bass_kernel_spmd
import ml_dtypes

F32 = mybir.dt.float32
BF16 = mybir.dt.bfloat16
ALU = mybir.AluOpType
AF = mybir.ActivationFunctionType
BF = ml_dtypes.bfloat16

B, C, L, H, W = 2, 64, 16, 96, 96
CO2 = 64
NCORES = 8
HB, HG = 12, 14
NPP = HG * W                      # 1344 gather positions per (plane, l)
CHUNKS = ((0, 3), (3, 3), (6, 3), (9, 3), (12, 2))
CMAX = 3 * 96
PAIRS = [(dz, dx) for dz in range(3) for dx in range(3)]  # dy in (0,1) pairs
P3GROUPS = ((0, 5), (5, 5), (10, 2))

_nc_cache = None


def build_program():
    nc = bacc.Bacc("TRN2", target_bir_lowering=False, debug=False,
                   num_devices=NCORES)
    p1slab = nc.dram_tensor("p1slab", [B, L, C, 3, HG, 6, 98], BF16,
                            kind="ExternalInput").ap()
    wstp = nc.dram_tensor("wstp", [L, 128, HG, 9, 64], BF16,
                          kind="ExternalInput").ap()
    wsts = nc.dram_tensor("wsts", [L, 64, HG, 9, 64], BF16,
                          kind="ExternalInput").ap()
    xwin = nc.dram_tensor("xwin", [B, C, 20, 18, 96], BF16,
                          kind="ExternalInput").ap()
    consts = nc.dram_tensor("consts", [128, 6, NPP], BF16,
                            kind="ExternalInput").ap()
    wcp = nc.dram_tensor("wcp", [128, 9, 64], BF16, kind="ExternalInput").ap()
    wcs = nc.dram_tensor("wcs", [64, 9, 64], BF16, kind="ExternalInput").ap()
    bconv = nc.dram_tensor("b_conv", [CO2, 1], F32, kind="ExternalInput").ap()
    out_ext = nc.dram_tensor("out", [B, CO2, L, HB, W], F32,
                             kind="ExternalOutput").ap()
    def_scr = nc.dram_tensor("def_scr", [B, C, L, HG, W], BF16).ap()

    with tile.TileContext(nc) as tc:
        with tc.tile_pool(name="pw", bufs=1) as pw, \
             tc.tile_pool(name="pl", bufs=2) as pl, \
             tc.tile_pool(name="plo", bufs=1) as plo, \
             tc.tile_pool(name="psl", bufs=1) as psl, \
             tc.tile_pool(name="ps1", bufs=4, space="PSUM") as ps1p, \
             tc.tile_pool(name="p2t", bufs=1) as p2t:
            cst = pw.tile([128, 6, NPP], BF16)
            nc.sync.dma_start(cst[:], consts)
            ones = pw.tile([128, 1], F32)
            nc.vector.memset(ones[:], 1.0)
            taps_r = cst[:, 5:6, 0:5]                                 # [128,1,5]
            taps_c = cst[:, 5:6, 0:5].rearrange("p a t -> p t a")     # [128,5,1]

            for l in range(L):
                # ---------- window for gather ----------
                win = pl.tile([128, 18, 5, 100], BF16, tag="win")
                nc.vector.memset(win[:].rearrange("p a b c -> p (a b c)"), 0.0)
                for zz in range(5):
                    nc.sync.dma_start(
                        win[:, :, zz, 2:98],
                        xwin[:, :, l + zz, :, :].rearrange(
                            "b c y x -> (b c) y x"))
                # ---------- P1: offset conv at scrambled triplets ----------
                wstpt = psl.tile([128, HG, 9, 64], BF16, tag="wstp")
                nc.sync.dma_start(wstpt[:], wstp[l])
                wstst = psl.tile([64, HG, 9, 64], BF16, tag="wsts")
                nc.sync.dma_start(wstst[:], wsts[l])
                offc = plo.tile([128, HG, 288], BF16, tag="offc")
                for b in range(B):
                    for half in range(2):
                        hs, hn = 7 * half, 7
                        st = pl.tile([128, 3, 7, 5, 98], BF16, tag="slab")
                        nc.sync.dma_start(
                            st[0:64], p1slab[b, l, :, :, hs:hs + hn, 0:5, :])
                        nc.sync.dma_start(
                            st[64:128], p1slab[b, l, :, :, hs:hs + hn, 1:6, :])
                        for hr in range(hn):
                            hh = hs + hr
                            ps = ps1p.tile([128, 288], F32, tag="ps1")
                            for i, (dz, dx) in enumerate(PAIRS):
                                nc.tensor.matmul(
                                    ps[64 * b:64 * b + 64], wstpt[:, hh, i, :],
                                    st[:, dz, hr, 0:3, dx:dx + 96],
                                    start=(i == 0), stop=False)
                            for i, (dz, dx) in enumerate(PAIRS):
                                nc.tensor.matmul(
                                    ps[64 * b:64 * b + 64], wstst[:, hh, i, :],
                                    st[0:64, dz, hr, 2:5, dx:dx + 96],
                                    start=False, stop=(i == 8))
                            nc.vector.tensor_copy(
                                offc[64 * b:64 * b + 64, hh, :],
                                ps[64 * b:64 * b + 64])

                # ---------- P2: tent-weight gather ----------
                for (c0, cn) in CHUNKS:
                    npc = cn * 96
                    sl = slice(c0 * 96, c0 * 96 + npc)
                    offv = offc[:, c0:c0 + cn, :].rearrange(
                        "p h (w k) -> p (h w) k", w=96, k=3)
                    az = p2t.tile([128, CMAX, 1], BF16, tag="az")
                    ay = p2t.tile([128, CMAX, 1], BF16, tag="ay")
                    ax = p2t.tile([128, CMAX, 1], BF16, tag="ax")
                    tv0 = p2t.tile([128, CMAX, 1], BF16, tag="tv0")
                    nc.vector.tensor_scalar(
                        az[:, :npc], offv[:, :, 0:1],
                        float(-l), float(15 - l), ALU.max, ALU.min)
                    nc.vector.tensor_tensor(
                        tv0[:, :npc], offv[:, :, 1:2], cst[:, 0, sl], ALU.max)
                    nc.vector.tensor_tensor(
                        ay[:, :npc], tv0[:, :npc], cst[:, 1, sl], ALU.min)
                    nc.vector.tensor_tensor(
                        tv0[:, :npc], offv[:, :, 2:3], cst[:, 2, sl], ALU.max)
                    nc.vector.tensor_tensor(
                        ax[:, :npc], tv0[:, :npc], cst[:, 3, sl], ALU.min)

                    ut = p2t.tile([128, CMAX, 5], BF16, tag="ut")
                    wt = p2t.tile([128, CMAX, 5], BF16, tag="wt")
                    lamx = p2t.tile([128, CMAX, 5], BF16, tag="lamx")
                    lamy = p2t.tile([128, CMAX, 5], BF16, tag="lamy")

                    for a_t, lam in ((ax, lamx), (ay, lamy)):
                        a_b, t_b = broadcast_tensor_aps(a_t[:, :npc], taps_r)
                        nc.vector.tensor_tensor(
                            ut[:, :npc], a_b, t_b, ALU.subtract)
                        nc.vector.scalar_tensor_tensor(
                            wt[:, :npc], ut[:, :npc], -1.0, ut[:, :npc],
                            ALU.mult, ALU.min)
                        nc.scalar.activation(
                            lam[:, :npc], wt[:, :npc], AF.Relu, bias=ones[:])

                    uzt = p2t.tile([128, 5, CMAX], BF16, tag="uzt")
                    wzt = p2t.tile([128, 5, CMAX], BF16, tag="wzt")
                    lamz = p2t.tile([128, 5, CMAX], BF16, tag="lamz")
                    az_r = az[:, :npc, 0:1].rearrange("p n o -> p o n")
                    a_b, t_b = broadcast_tensor_aps(az_r, taps_c)
                    nc.vector.tensor_tensor(
                        uzt[:, :, :npc], a_b, t_b, ALU.subtract)
                    nc.vector.scalar_tensor_tensor(
                        wzt[:, :, :npc], uzt[:, :, :npc], -1.0,
                        uzt[:, :, :npc], ALU.mult, ALU.min)
                    nc.scalar.activation(
                        lamz[:, :, :npc], wzt[:, :, :npc], AF.Relu,
                        bias=ones[:])

                    lyx = p2t.tile([128, CMAX, 5, 5], BF16, tag="lyx")
                    ly_b = lamy[:, :npc].unsqueeze(3).broadcast_to(
                        [128, npc, 5, 5])
                    lx_b = lamx[:, :npc].unsqueeze(2).broadcast_to(
                        [128, npc, 5, 5])
                    nc.vector.tensor_tensor(lyx[:, :npc], ly_b, lx_b, ALU.mult)

                    prod = p2t.tile([128, CMAX, 5, 5], BF16, tag="prod")
                    red = p2t.tile([128, CMAX], F32, tag="red")
                    acc = p2t.tile([128, CMAX], F32, tag="acc")
                    tvf = p2t.tile([128, CMAX], F32, tag="tvf")
                    accb = p2t.tile([128, CMAX], BF16, tag="accb")
                    for szi in range(5):
                        for syi in range(5):
                            s = win[:, c0 + syi, szi, 0:96]
                            v = AP(s.tensor, s.offset,
                                   [s.ap[0], (500, cn), (1, 96), (1, 5)])
                            nc.vector.tensor_tensor(
                                prod[:, :npc, syi, :], lyx[:, :npc, syi, :],
                                v, ALU.mult)
                        nc.vector.tensor_reduce(
                            red[:, :npc], prod[:, :npc],
                            axis=mybir.AxisListType.XY, op=ALU.add)
                        if szi == 0:
                            nc.vector.tensor_tensor(
                                acc[:, :npc], red[:, :npc],
                                lamz[:, szi, :npc], ALU.mult)
                        else:
                            nc.vector.tensor_tensor(
                                tvf[:, :npc], red[:, :npc],
                                lamz[:, szi, :npc], ALU.mult)
                            nc.vector.tensor_tensor(
                                acc[:, :npc], acc[:, :npc], tvf[:, :npc],
                                ALU.add)
                    nc.vector.tensor_tensor(
                        accb[:, :npc], acc[:, :npc], cst[:, 4, sl], ALU.mult)
                    for b in range(B):
                        nc.sync.dma_start(
                            def_scr[b, :, l, c0:c0 + cn, :].rearrange(
                                "c h x -> c (h x)"),
                            accb[64 * b:64 * b + 64, :npc])

        # ---------------- P3: main conv + bias ----------------
        with tc.tile_pool(name="p3w", bufs=1) as p3w, \
             tc.tile_pool(name="p3l", bufs=1) as p3l, \
             tc.tile_pool(name="ps3", bufs=2, space="PSUM") as ps3p, \
             tc.tile_pool(name="p3o", bufs=3) as p3o:
            wcpt = p3w.tile([128, 9, 64], BF16)
            nc.sync.dma_start(wcpt[:], wcp)
            wcst = p3w.tile([64, 9, 64], BF16)
            nc.sync.dma_start(wcst[:], wcs)
            bct = p3w.tile([CO2, 1], F32)
            nc.sync.dma_start(bct[:], bconv)
            for b in range(B):
                dc = p3l.tile([128, 18, 16, 98], BF16, tag="dc")
                nc.vector.memset(dc[:].rearrange("p a b c -> p (a b c)"), 0.0)
                for z in range(L):
                    nc.sync.dma_start(dc[0:64, 1 + z, 1:15, 1:97],
                                      def_scr[b, :, z])
                    nc.sync.dma_start(dc[64:128, 1 + z, 0:14, 1:97],
                                      def_scr[b, :, z])
                for l in range(L):
                    for (hc0, hcn) in P3GROUPS:
                        nmm = hcn * W
                        ps = ps3p.tile([64, 480], F32, tag="ps3")
                        for i, (dz, dx) in enumerate(PAIRS):
                            nc.tensor.matmul(
                                ps[:, :nmm], wcpt[:, i, :],
                                dc[:, l + dz, hc0 + 1:hc0 + 1 + hcn,
                                   dx:dx + 96],
                                start=(i == 0), stop=False)
                        for i, (dz, dx) in enumerate(PAIRS):
                            nc.tensor.matmul(
                                ps[:, :nmm], wcst[:, i, :],
                                dc[0:64, l + dz, hc0 + 3:hc0 + 3 + hcn,
                                   dx:dx + 96],
                                start=False, stop=(i == 8))
                        ob = p3o.tile([64, 480], F32, tag="ob")
                        nc.vector.tensor_scalar(
                            ob[:, :nmm], ps[:, :nmm], bct[:], None, ALU.add)
                        nc.sync.dma_start(
                            out_ext[b, :, l, hc0:hc0 + hcn, :].rearrange(
                                "m h x -> m (h x)"),
                            ob[:, :nmm])
    nc.finalize()
    return nc


# ---------------- host-side index tables (static) ----------------
def _build_tables():
    tables = []
    for k in range(NCORES):
        h0 = 12 * k - 1
        PI = np.zeros((L, 3, HG, 6), np.int64)
        RHO = np.zeros((L, 3, HG, 6), np.int64)
        QM = np.zeros((L, HG), np.int64)
        for l in range(L):
            for hh in range(HG):
                R = 96 * l + h0 + hh
                if not (0 <= R < 1536):
                    continue                      # masked: slab stays zeros
                q = R // 512
                rr = 3 * (R % 512)
                pi, rho = rr // 96, rr % 96       # plane, row of triplet start
                QM[l, hh] = q
                for zz in range(3):
                    zi = pi - 1 + zz              # -1..16
                    for u in range(6):
                        yi = rho - 1 + u          # -1..97
                        PI[l, zz, hh, u] = zi + 1     # into padded 18
                        RHO[l, zz, hh, u] = yi + 1    # into padded 98
        tables.append((PI, RHO, QM))
    return tables


_TABLES = _build_tables()


def _host_prep(x, w_off, w_conv, b_conv):
    xbf = x.astype(BF)
    # padded source volume for P1 slab gather: z pad 1 each side, y pad 1
    xpad = np.zeros((B, C, 18, 99, 96), BF)
    xpad[:, :, 1:17, 1:97, :] = xbf
    xflat = np.ascontiguousarray(xpad.reshape(B, C, 18 * 99, 96))

    # P2 window source: z pad 2 each side, y rows per core handled below
    xz2 = np.zeros((B, C, 20, H, W), BF)
    xz2[:, :, 2:18] = xbf

    # stationary weights for P1 (3 q-variants) and P3
    w_off_r = w_off.reshape(3 * C, C, 3, 3, 3).astype(BF)
    # wpair[q, 64*h + c, p=(dz,dx), m] = w_off[3m+q, c, dz, h, dx]
    wpair = np.empty((3, 128, 9, 64), BF)
    wsing = np.empty((3, 64, 9, 64), BF)
    for q in range(3):
        wq = w_off_r[q::3]                        # [64(m), C, 3, 3, 3]
        for p, (dz, dx) in enumerate(PAIRS):
            wpair[q, 0:64, p] = wq[:, :, dz, 0, dx].T
            wpair[q, 64:128, p] = wq[:, :, dz, 1, dx].T
            wsing[q, :, p] = wq[:, :, dz, 2, dx].T
    wconv_bf = w_conv.astype(BF)
    wcp = np.empty((128, 9, 64), BF)
    wcs = np.empty((64, 9, 64), BF)
    for p, (dz, dx) in enumerate(PAIRS):
        wcp[0:64, p] = wconv_bf[:, :, dz, 0, dx].T
        wcp[64:128, p] = wconv_bf[:, :, dz, 1, dx].T
        wcs[:, p] = wconv_bf[:, :, dz, 2, dx].T
    bc = np.ascontiguousarray(b_conv.reshape(CO2, 1)).astype(np.float32)

    gx = np.tile(np.arange(W, dtype=np.float32), HG)
    in_maps = []
    for k in range(NCORES):
        PI, RHO, QM = _TABLES[k]
        fidx = (PI * 99 + RHO).ravel()
        slab6 = np.take(xflat, fidx, axis=2).reshape(
            B, C, L, 3, HG, 6, 96)
        p1slab = np.zeros((B, L, C, 3, HG, 6, 98), BF)
        p1slab[..., 1:97] = slab6.transpose(0, 2, 1, 3, 4, 5, 6)

        wstp = np.ascontiguousarray(
            wpair[QM].transpose(0, 2, 1, 3, 4))    # [L, 128, HG, 9, 64]
        wsts = np.ascontiguousarray(
            wsing[QM].transpose(0, 2, 1, 3, 4))    # [L, 64, HG, 9, 64]

        # gather window rows: global 12k-3 .. 12k+14 (18 rows)
        xw = np.zeros((B, C, 20, 18, 96), BF)
        r0 = 12 * k - 3
        lo, hi = max(0, r0), min(H, r0 + 18)
        xw[:, :, :, lo - r0:hi - r0, :] = xz2[:, :, :, lo:hi, :]

        # clamp bounds + mask + taps
        h0 = 12 * k - 1
        hvec = np.repeat(np.arange(HG, dtype=np.float32) + h0, W)
        valid = (hvec >= 0) & (hvec <= 95)
        lo_y = np.where(valid, -hvec, 0.0)
        hi_y = np.where(valid, 95.0 - hvec, 0.0)
        consts = np.zeros((128, 6, NPP), BF)
        consts[:, 0] = lo_y.astype(BF)
        consts[:, 1] = hi_y.astype(BF)
        consts[:, 2] = (-gx).astype(BF)
        consts[:, 3] = (95.0 - gx).astype(BF)
        consts[:, 4] = valid.astype(np.float32).astype(BF)
        consts[:, 5, 0:5] = np.arange(-2, 3, dtype=np.float32).astype(BF)

        in_maps.append({
            "p1slab": p1slab, "wstp": wstp, "wsts": wsts, "xwin": xw,
            "consts": consts, "wcp": wcp, "wcs": wcs, "b_conv": bc,
        })
    return in_maps


def kernel(x, w_off, w_conv, b_conv):
    global _nc_cache
    x = np.asarray(x, dtype=np.float32)
    w_off = np.asarray(w_off, dtype=np.float32)
    w_conv = np.asarray(w_conv, dtype=np.float32)
    b_conv = np.asarray(b_conv, dtype=np.float32)

    if _nc_cache is None:
        _nc_cache = build_program()

    in_maps = _host_prep(x, w_off, w_conv, b_conv)
    res = run_bass_kernel_spmd(_nc_cache, in_maps, list(range(NCORES)))
    out = np.empty((B, CO2, L, H, W), np.float32)
    for k in range(NCORES):
        out[:, :, :, 12 * k:12 * k + HB, :] = res.results[k]["out"]
    return out
